# revision 1
# baseline (speedup 1.0000x reference)
"""Trainium2 Bass kernel for the attention-scoring MLP (nn_Attn):

    enc = encoder_outputs.transpose(1,0,2)          # [B,S,Hin]
    a1  = tanh(enc @ W1_enc.T + hidden @ W1_hid.T + b1)
    s   = a1 @ W2[0] (+ b2 -- dropped: softmax shift-invariant)
    s   = where(mask, -inf, s)
    out = softmax(s, axis=-1)[:, None, :]           # [B,1,S]

Sharding: data-parallel over batch B=32 across 8 NeuronCores (4 rows
each), weights replicated, no collectives. Per core the main matmul is
computed transposed -- a1T[h, s] = W1_encT.T @ encT per batch row -- so
the (b1 + hidden@W1_hid.T) term rides the ScalarEngine's per-partition
bias port of the tanh activation, and the W2 contraction is a
PSUM-accumulated M=1 matmul over h-tiles. Matmuls run in bf16 (inputs
pre-transposed and converted host-side so all DMAs are contiguous
row-major loads); accumulation is fp32 in PSUM.
"""

import numpy as np
import ml_dtypes

import concourse.bass as bass
import concourse.tile as tile
from concourse import bacc, mybir
from concourse.bass import ds, ts
from concourse.bass_utils import run_bass_kernel_spmd
from concourse.masks import make_identity

N_CORES = 8
B, S, HIN, H = 32, 1024, 1024, 1024
BL = B // N_CORES          # local batch rows per core
P = 128                    # partitions
IT = HIN // P              # contraction tiles
HT = H // P                # output-feature tiles
NT = 512                   # moving-dim tile (s columns per matmul)
SH = S // NT               # s tiles per batch row
F32 = mybir.dt.float32
BF16 = mybir.dt.bfloat16
AF = mybir.ActivationFunctionType
BF = ml_dtypes.bfloat16

_cached_nc = None
LAST_RESULT = None  # BassKernelResults of the most recent run (for test harness)


def _build():
    global _cached_nc
    if _cached_nc is not None:
        return _cached_nc

    nc = bacc.Bacc("TRN2", target_bir_lowering=False, debug=False,
                   num_devices=N_CORES)

    # encT per batch row: [b, i, s]
    enc_ext = nc.dram_tensor("enc", [BL, HIN, S], BF16, kind="ExternalInput").ap()
    # hiddenT: [i, b]
    hidt_ext = nc.dram_tensor("hiddent", [H, BL], BF16, kind="ExternalInput").ap()
    mneg_ext = nc.dram_tensor("maskneg", [BL * S], F32, kind="ExternalInput").ap()
    # W1 split + transposed: [i, h]
    w1e_ext = nc.dram_tensor("w1e", [HIN, H], BF16, kind="ExternalInput").ap()
    w1h_ext = nc.dram_tensor("w1h", [H, H], BF16, kind="ExternalInput").ap()
    b1_ext = nc.dram_tensor("b1", [H], F32, kind="ExternalInput").ap()
    w2_ext = nc.dram_tensor("w2", [H], BF16, kind="ExternalInput").ap()
    out_ext = nc.dram_tensor("out", [BL, S], F32, kind="ExternalOutput").ap()

    with tile.TileContext(nc) as tc:
        with (
            tc.tile_pool(name="consts", bufs=1) as consts,
            tc.tile_pool(name="encp", bufs=3) as encp,
            tc.tile_pool(name="thp", bufs=7) as thp,
            tc.tile_pool(name="pap", bufs=2, space="PSUM") as pap,
            tc.tile_pool(name="pscp", bufs=2, space="PSUM") as pscp,
            tc.tile_pool(name="psA", bufs=1, space="PSUM") as psA,
            tc.tile_pool(name="psT", bufs=2, space="PSUM") as psTp,
        ):
            # ---- PE warmup: ~4us of junk matmuls with no DMA deps so the
            # HAM clock-gate is already at 8/8 when the real matmuls arrive.
            warm_sb = consts.tile([P, NT], BF16)
            nc.gpsimd.memset(warm_sb[:], 0.0)
            warm_ps = pap.tile([P, NT], F32, tag="pa1")
            for _ in range(10):
                nc.tensor.matmul(warm_ps[:], warm_sb[:, 0:P], warm_sb[:],
                                 start=True, stop=True)

            # ---- resident weights/constants ----
            # DMA emission order = ring service order: first-needed first.
            # w1e_sb[p, it*H + h] = W1[h, it*128+p]  == w1e_ext[it*128+p, h]
            # One DMA per h-tile: the ht=0 matmul group only waits for 256KB
            # of weights instead of the whole 2MB.
            w1e_t = []
            for it in range(IT):
                w = consts.tile([P, H], BF16, tag=f"w1e{it}")
                nc.sync.dma_start(w[:], w1e_ext[ds(it * P, P), :])
                w1e_t.append(w)
            hT_sb = consts.tile([P, IT * BL], BF16)
            for it in range(IT):
                nc.sync.dma_start(hT_sb[:, ts(it, BL)], hidt_ext[ds(it * P, P), :])
            # first enc block is prefetched here, before w1h (phase A can
            # wait). Split into per-it tiles so the very first matmul only
            # needs w1e[0]+enc0[0] (~256KB), not the whole 3MB preload.
            enc0_t = []
            for it in range(IT):
                e = encp.tile([P, NT], BF16, tag=f"enc0_{it}")
                nc.scalar.dma_start(e[:], enc_ext[0, ds(it * P, P), ds(0, NT)])
                enc0_t.append(e)
            w1h_t = []
            for it in range(IT):
                w = consts.tile([P, H], BF16, tag=f"w1h{it}")
                nc.scalar.dma_start(w[:], w1h_ext[ds(it * P, P), :])
                w1h_t.append(w)
            b1T_sb = consts.tile([P, HT], F32)
            nc.sync.dma_start(b1T_sb[:], b1_ext.rearrange("(ht p) -> p ht", p=P))
            w2T_sb = consts.tile([P, HT], BF16)
            nc.sync.dma_start(w2T_sb[:], w2_ext.rearrange("(ht p) -> p ht", p=P))
            mneg_sb = consts.tile([1, BL * S], F32)
            nc.sync.dma_start(mneg_sb[:], mneg_ext[:])
            ident_sb = consts.tile([BL, BL], F32)
            make_identity(nc, ident_sb[:])
            # W2 as a padded [128,128] stationary per h-tile (column 0 = w2
            # chunk, rest zero) so the scores matmul keeps the same PE config
            # as the main matmuls; only row 0 of its PSUM output is used.
            w2pad = consts.tile([P, HT * P], BF16)
            nc.gpsimd.memset(w2pad[:], 0.0)
            for ht in range(HT):
                nc.vector.tensor_copy(w2pad[:, ds(ht * P, 1)], w2T_sb[:, ds(ht, 1)])

            bias_sb = consts.tile([P, HT * BL], F32)   # [p, ht*BL+b]
            hterm_sb = consts.tile([BL, H], F32)
            scores_sb = consts.tile([1, BL * S], F32)
            c40 = consts.tile([1, 1], F32)
            nc.gpsimd.memset(c40[:], -40.0)
            exps = consts.tile([1, BL * S], F32)
            ssum = consts.tile([1, BL * SH], F32)
            rcp = consts.tile([1, BL], F32)
            attn = consts.tile([1, BL * S], F32)

            # ---- phase A: h_term[b,h] = hidden @ W1_hid.T; bias = h_termT + b1T
            pht = psA.tile([BL, H], F32)
            for it in range(IT):
                lhs = hT_sb[:, ts(it, BL)]
                nc.tensor.matmul(pht[:, 0:NT], lhs,
                                 w1h_t[it][:, ds(0, NT)],
                                 start=(it == 0), stop=(it == IT - 1))
                nc.tensor.matmul(pht[:, NT:H], lhs,
                                 w1h_t[it][:, ds(NT, NT)],
                                 start=(it == 0), stop=(it == IT - 1))
            nc.scalar.copy(hterm_sb[:], pht[:])
            for ht in range(HT):
                ptT = psTp.tile([P, BL], F32)
                nc.tensor.transpose(ptT[:], hterm_sb[:, ts(ht, P)], ident_sb[:])
                nc.vector.tensor_scalar_add(bias_sb[:, ts(ht, BL)], ptT[:],
                                            b1T_sb[:, ds(ht, 1)])

            # ---- phase B: per (b, s-half) tile of 512 sequence positions
            for t in range(BL * SH):
                b, sh = divmod(t, SH)
                # encT block: enc_sb[p, it*NT + s] = enc_ext[b, it*128+p, sh*NT+s]
                if t == 0:
                    enc_sb = None
                else:
                    enc_sb = encp.tile([P, IT * NT], BF16, tag="enc")
                    # t==1 rides the scalar ring (startup overlap with w1e on
                    # sync); steady-state tiles use the otherwise-idle sync
                    # ring so DMA triggers never serialize against tanh on ACT.
                    eng = nc.scalar if t == 1 else nc.sync
                    for it in range(IT):
                        eng.dma_start(
                            enc_sb[:, ts(it, NT)],
                            enc_ext[b, ds(it * P, P), ds(sh * NT, NT)],
                        )
                psc = pscp.tile([P, NT], F32)
                # Delay the scores matmuls so a late bias (phase A is still
                # streaming during t=0) never stalls the in-order PE.
                delay = 4 if t == 0 else (1 if t == BL * SH - 1 else 3)
                pending = []
                for ht in range(HT):
                    pa1 = pap.tile([P, NT], F32, tag="pa1")
                    for it in range(IT):
                        rhs = enc0_t[it][:] if t == 0 else enc_sb[:, ts(it, NT)]
                        nc.tensor.matmul(
                            pa1[:],
                            w1e_t[it][:, ds(ht * P, P)],
                            rhs,
                            start=(it == 0), stop=(it == IT - 1),
                        )
                    th = thp.tile([P, NT], BF16)
                    nc.scalar.activation(th[:], pa1[:], AF.Tanh,
                                         bias=bias_sb[:, ds(ht * BL + b, 1)],
                                         scale=1.0)
                    pending.append((th, ht))
                    if len(pending) > delay:
                        pth, pht_idx = pending.pop(0)
                        nc.tensor.matmul(psc[:], w2pad[:, ds(pht_idx * P, P)],
                                         pth[:],
                                         start=(pht_idx == 0),
                                         stop=(pht_idx == HT - 1))
                for pth, pht_idx in pending:
                    nc.tensor.matmul(psc[:], w2pad[:, ds(pht_idx * P, P)],
                                     pth[:], start=(pht_idx == 0),
                                     stop=(pht_idx == HT - 1))
                # scores += mask * -1e30   (scores_sb[0, t*NT:] == scores[b, sh*NT:])
                nc.vector.tensor_add(scores_sb[0:1, ds(t * NT, NT)], psc[0:1, :],
                                     mneg_sb[0:1, ds(t * NT, NT)])

                # ---- softmax, pipelined per s-half tile.
                # |scores| <= ||W2||_1 <= 32, so exp(s - 40) never overflows
                # and softmax is shift-invariant -- no max-reduce needed.
                nc.scalar.activation(exps[0:1, ds(t * NT, NT)],
                                     scores_sb[0:1, ds(t * NT, NT)],
                                     AF.Exp, bias=c40[0:1, 0:1], scale=1.0,
                                     accum_out=ssum[0:1, ds(t, 1)])
                if sh == SH - 1:
                    # total = sum of the SH per-tile partial sums for row b
                    nc.vector.reduce_sum(rcp[0:1, ds(b, 1)],
                                         ssum[0:1, ds(b * SH, SH)],
                                         axis=mybir.AxisListType.X)
                    nc.vector.reciprocal(rcp[0:1, ds(b, 1)], rcp[0:1, ds(b, 1)])
                    nc.vector.tensor_scalar_mul(attn[0:1, ds(b * S, S)],
                                                exps[0:1, ds(b * S, S)],
                                                rcp[0:1, ds(b, 1)])
                    nc.sync.dma_start(out_ext[b, :], attn[0:1, ds(b * S, S)])

    nc.compile()
    _cached_nc = nc
    return nc


def kernel(hidden, encoder_outputs, mask, W1, b1, W2, b2):
    global LAST_RESULT
    nc = _build()

    enc = np.asarray(encoder_outputs, dtype=np.float32)
    # [S,B,Hin] -> [B,Hin,S] in bf16 so per-core DMAs are contiguous
    enc_t = np.ascontiguousarray(np.transpose(enc, (1, 2, 0)).astype(BF))
    hid_t = np.ascontiguousarray(np.asarray(hidden, dtype=np.float32).T.astype(BF))  # [H,B]
    maskneg = np.where(np.asarray(mask, dtype=bool), np.float32(-1e30),
                       np.float32(0.0)).astype(np.float32)
    W1 = np.asarray(W1, dtype=np.float32)
    w1e = np.ascontiguousarray(W1[:, :HIN].T.astype(BF))   # [Hin, H]
    w1h = np.ascontiguousarray(W1[:, HIN:].T.astype(BF))   # [H, H]
    b1 = np.ascontiguousarray(np.asarray(b1, dtype=np.float32).reshape(H))
    w2 = np.ascontiguousarray(np.asarray(W2, dtype=np.float32).reshape(H).astype(BF))

    in_maps = []
    for c in range(N_CORES):
        sl = slice(c * BL, (c + 1) * BL)
        in_maps.append({
            "enc": np.ascontiguousarray(enc_t[sl]),
            "hiddent": np.ascontiguousarray(hid_t[:, sl]),
            "maskneg": np.ascontiguousarray(maskneg[sl].reshape(-1)),
            "w1e": w1e,
            "w1h": w1h,
            "b1": b1,
            "w2": w2,
        })

    res = run_bass_kernel_spmd(nc, in_maps, core_ids=list(range(N_CORES)))
    LAST_RESULT = res
    out = np.concatenate([res.results[c]["out"] for c in range(N_CORES)], axis=0)
    return np.ascontiguousarray(out[:, None, :].astype(np.float32))



# revision 19
# speedup vs baseline: 1.5363x; 1.5363x over previous
"""Trainium2 Bass kernel for the attention-scoring MLP (nn_Attn):

    enc = encoder_outputs.transpose(1,0,2)          # [B,S,Hin]
    a1  = tanh(enc @ W1_enc.T + hidden @ W1_hid.T + b1)
    s   = a1 @ W2[0] (+ b2 -- dropped: softmax shift-invariant)
    s   = where(mask, -inf, s)
    out = softmax(s, axis=-1)[:, None, :]           # [B,1,S]

Sharding: data-parallel over batch B=32 across 8 NeuronCores (4 rows
each), weights replicated, no collectives. Per core the main matmul is
computed transposed -- a1T[h, s] = W1_encT.T @ encT per batch row -- so
the (b1 + hidden@W1_hid.T) term rides the ScalarEngine's per-partition
bias port of the tanh activation, and the W2 contraction is a
PSUM-accumulated M=1 matmul over h-tile pairs.

All heavy matmuls run in fp8 (e4m3) with MatmulPerfMode.DoubleRow: each
matmul contracts a PAIR of 128-row k-tiles (lhsT/rhs laid out [128, 2,
n]), which the PE streams at 2 fp8 rows/cycle. Weights are pre-scaled
by 32 host-side so their +-1/sqrt(2048) range sits in fp8's normal
range; the 1/32 is folded into the tanh / exp activation scale.
Accumulation is fp32 in PSUM.
"""

import numpy as np
import ml_dtypes

import concourse.bass as bass
import concourse.tile as tile
from concourse import bacc, mybir
from concourse.bass import ds, ts
from concourse.bass_utils import run_bass_kernel_spmd
from concourse.masks import make_identity

N_CORES = 8
B, S, HIN, H = 32, 1024, 1024, 1024
BL = B // N_CORES          # local batch rows per core
P = 128                    # partitions
IT = HIN // P              # contraction k-tiles
KP = IT // 2               # k-tile pairs (DoubleRow)
HT = H // P                # output-feature tiles
NT = 512                   # moving-dim tile (s columns per matmul)
SH = S // NT               # s tiles per batch row
WS = 32.0                  # host-side weight scale (undone in act scale)
F32 = mybir.dt.float32
BF16 = mybir.dt.bfloat16
FP8 = mybir.dt.float8e4
AF = mybir.ActivationFunctionType
DR = mybir.MatmulPerfMode.DoubleRow
F8 = ml_dtypes.float8_e4m3

_cached_nc = None
LAST_RESULT = None  # BassKernelResults of the most recent run (for test harness)


def _build():
    global _cached_nc
    if _cached_nc is not None:
        return _cached_nc

    nc = bacc.Bacc("TRN2", target_bir_lowering=False, debug=False,
                   num_devices=N_CORES)

    # encT per batch row: [b, k, s] fp8
    enc_ext = nc.dram_tensor("enc", [BL, HIN, S], FP8, kind="ExternalInput").ap()
    # hiddenT packed [p, it, b] fp8 (k = it*128+p)
    hidt_ext = nc.dram_tensor("hiddent", [P, IT, BL], FP8, kind="ExternalInput").ap()
    mneg_ext = nc.dram_tensor("maskneg", [BL * S], F32, kind="ExternalInput").ap()
    # W1_enc.T packed per ht: [ht, p, it, m] = 32*W1[ht*128+m, it*128+p]
    w1e_ext = nc.dram_tensor("w1e", [HT, P, IT, P], FP8, kind="ExternalInput").ap()
    # W1_hid.T packed: [p, it*H + h] = 32*W1[HIN + h ... ] (see host packing)
    w1h_ext = nc.dram_tensor("w1h", [P, IT * H], FP8, kind="ExternalInput").ap()
    b1_ext = nc.dram_tensor("b1", [H], F32, kind="ExternalInput").ap()
    # W2 packed [p, ht] f32 (h = ht*128+p) -- per-partition scalars for the
    # DVE score accumulation
    w2_ext = nc.dram_tensor("w2", [P, HT], F32, kind="ExternalInput").ap()
    out_ext = nc.dram_tensor("out", [BL, S], F32, kind="ExternalOutput").ap()

    with tile.TileContext(nc) as tc:
        with (
            tc.tile_pool(name="consts", bufs=1) as consts,
            tc.tile_pool(name="encp", bufs=3) as encp,
            tc.tile_pool(name="thp", bufs=4) as thp,
            tc.tile_pool(name="accp", bufs=3) as accp,
            tc.tile_pool(name="pap", bufs=2, space="PSUM") as pap,
            tc.tile_pool(name="pscp", bufs=2, space="PSUM") as pscp,
            tc.tile_pool(name="psA", bufs=1, space="PSUM") as psA,
            tc.tile_pool(name="psT", bufs=2, space="PSUM") as psTp,
        ):
            # ---- PE warmup: junk matmuls with no DMA deps so the HAM
            # clock-gate is ramping toward 8/8 when the real matmuls arrive.
            warm_sb = consts.tile([P, NT], BF16)
            nc.gpsimd.memset(warm_sb[:], 0.0)
            warm_ps = pap.tile([P, NT], F32, tag="pa1")
            for _ in range(10):
                nc.tensor.matmul(warm_ps[:], warm_sb[:, 0:P], warm_sb[:],
                                 start=True, stop=True)

            # ---- resident weights/constants ----
            # DMA emission order = ring service order: first-needed first.
            # sync ring: hidden (tiny) -> w1e per-ht -> small consts.
            hT_sb = consts.tile([P, IT, BL], FP8)
            nc.sync.dma_start(hT_sb[:], hidt_ext[:])
            w1e_t = []
            for ht in range(HT):
                w = consts.tile([P, IT, P], FP8, tag=f"w1e{ht}")
                nc.sync.dma_start(w[:], w1e_ext[ht])
                w1e_t.append(w)
            b1T_sb = consts.tile([P, HT], F32)
            nc.sync.dma_start(b1T_sb[:], b1_ext.rearrange("(ht p) -> p ht", p=P))
            w2T_sb = consts.tile([P, HT], F32)
            nc.sync.dma_start(w2T_sb[:], w2_ext[:])
            mneg_sb = consts.tile([1, BL * S], F32)
            nc.sync.dma_start(mneg_sb[:], mneg_ext[:])
            # scalar ring: w1h (needed by phase A, overlaps warmup) then the
            # first enc block.
            w1h_sb = consts.tile([P, IT, H], FP8)
            for it in range(IT):
                nc.scalar.dma_start(w1h_sb[:, it, :], w1h_ext[:, ds(it * H, H)])
            enc0_sb = encp.tile([P, IT, NT], FP8, tag="enc")
            for it in range(IT):
                nc.scalar.dma_start(enc0_sb[:, it, :],
                                    enc_ext[0, ds(it * P, P), ds(0, NT)])
            ident_sb = consts.tile([BL, BL], F32)
            make_identity(nc, ident_sb[:])
            ones_sb = consts.tile([P, 1], BF16)
            nc.gpsimd.memset(ones_sb[:], 1.0)

            bias_sb = consts.tile([P, HT * BL], F32)   # [p, ht*BL+b]
            hterm_sb = consts.tile([BL, H], F32)
            scores_sb = consts.tile([1, BL * S], F32)
            c40 = consts.tile([1, 1], F32)
            nc.gpsimd.memset(c40[:], -40.0)
            exps = consts.tile([1, BL * S], F32)
            ssum = consts.tile([1, BL * SH], F32)
            rcp = consts.tile([1, BL], F32)
            attn = consts.tile([1, BL * S], F32)

            # ---- phase A: h_term[b,h] = hidden @ W1_hid.T (x32 in fp8);
            # bias = h_termT/32 + b1T. Plain fp8 matmuls (no DoubleRow: the
            # dual-fp8 ldweights ISA check needs pair-dim step%16==0, and
            # these stationaries are tiny anyway).
            pht = psA.tile([BL, H], F32)
            for g in range(2):
                for it in range(IT):
                    nc.tensor.matmul(pht[:, ds(g * NT, NT)],
                                     hT_sb[:, it, :],
                                     w1h_sb[:, it, ds(g * NT, NT)],
                                     start=(it == 0), stop=(it == IT - 1))
            nc.scalar.mul(hterm_sb[:], pht[:], 1.0 / WS)
            for ht in range(HT):
                ptT = psTp.tile([P, BL], F32)
                nc.tensor.transpose(ptT[:], hterm_sb[:, ts(ht, P)], ident_sb[:])
                nc.vector.tensor_scalar_add(bias_sb[:, ts(ht, BL)], ptT[:],
                                            b1T_sb[:, ds(ht, 1)])

            # ---- phase B: per (b, s-half) tile of 512 sequence positions.
            # Scores: the w2 contraction runs as a DVE accumulate chain over
            # ht (acc = w2[:,ht]*th_ht + acc, per-partition scalars) and a
            # single bf16 ones-matmul reduces the 128 partitions, keeping the
            # PE free for the DoubleRow main matmuls (whose dual-fp8
            # ldweights would also reject the tiny w2 stationary).
            def flush_scores(tp, acc_f):
                # softmax for tile tp, pipelined: |scores| <= ||W2||_1 <= 32,
                # so exp(s - 40) never overflows and softmax is
                # shift-invariant -- no max-reduce needed.
                bp, shp = divmod(tp, SH)
                psc = pscp.tile([1, NT], F32, tag="psc", name="psc")
                nc.tensor.matmul(psc[:], ones_sb[:], acc_f[:],
                                 start=True, stop=True)
                # scores += mask * -1e30
                nc.vector.tensor_add(scores_sb[0:1, ds(tp * NT, NT)],
                                     psc[0:1, :],
                                     mneg_sb[0:1, ds(tp * NT, NT)])
                nc.scalar.activation(exps[0:1, ds(tp * NT, NT)],
                                     scores_sb[0:1, ds(tp * NT, NT)],
                                     AF.Exp, bias=c40[0:1, 0:1], scale=1.0,
                                     accum_out=ssum[0:1, ds(tp, 1)])
                if shp == SH - 1:
                    # total = sum of the SH per-tile partial sums for row bp
                    nc.vector.reduce_sum(rcp[0:1, ds(bp, 1)],
                                         ssum[0:1, ds(bp * SH, SH)],
                                         axis=mybir.AxisListType.X)
                    nc.vector.reciprocal(rcp[0:1, ds(bp, 1)],
                                         rcp[0:1, ds(bp, 1)])
                    nc.vector.tensor_scalar_mul(attn[0:1, ds(bp * S, S)],
                                                exps[0:1, ds(bp * S, S)],
                                                rcp[0:1, ds(bp, 1)])
                    nc.sync.dma_start(out_ext[bp, :], attn[0:1, ds(bp * S, S)])

            prev = None  # (tile idx, final acc tile) awaiting score flush
            for t in range(BL * SH):
                b, sh = divmod(t, SH)
                if t == 0:
                    enc_sb = enc0_sb
                else:
                    enc_sb = encp.tile([P, IT, NT], FP8, tag="enc")
                    # t==1 rides the scalar ring (startup overlap with w1e on
                    # sync); steady-state tiles use the otherwise-idle sync
                    # ring so DMA triggers never serialize against tanh on ACT.
                    eng = nc.scalar if t == 1 else nc.sync
                    for it in range(IT):
                        eng.dma_start(
                            enc_sb[:, it, :],
                            enc_ext[b, ds(it * P, P), ds(sh * NT, NT)],
                        )
                acc = None
                for ht in range(HT):
                    pa1 = pap.tile([P, NT], F32, tag="pa1")
                    for k in range(KP):
                        nc.tensor.matmul(
                            pa1[:],
                            w1e_t[ht][:, ds(2 * k, 2), :],
                            enc_sb[:, ds(2 * k, 2), :],
                            start=(k == 0), stop=(k == KP - 1),
                            perf_mode=DR,
                        )
                    th = thp.tile([P, NT], BF16, tag="th", name="th")
                    nc.scalar.activation(th[:], pa1[:], AF.Tanh,
                                         bias=bias_sb[:, ds(ht * BL + b, 1)],
                                         scale=1.0 / WS)
                    nxt = accp.tile([P, NT], BF16, tag="acc", name="acc")
                    if ht == 0:
                        nc.vector.tensor_scalar_mul(nxt[:], th[:],
                                                    w2T_sb[:, ds(0, 1)])
                    else:
                        nc.vector.scalar_tensor_tensor(
                            nxt[:], th[:], w2T_sb[:, ds(ht, 1)], acc[:],
                            mybir.AluOpType.mult, mybir.AluOpType.add)
                    acc = nxt
                    # Flush the previous tile's scores once this tile's PE
                    # pipeline is 3 groups deep (never stalls the in-order
                    # PE).
                    if ht == 2 and prev is not None:
                        flush_scores(*prev)
                        prev = None
                prev = (t, acc)
            flush_scores(*prev)

    nc.compile()
    _cached_nc = nc
    return nc


def kernel(hidden, encoder_outputs, mask, W1, b1, W2, b2):
    global LAST_RESULT
    nc = _build()

    enc = np.asarray(encoder_outputs, dtype=np.float32)
    # [S,B,Hin] -> [B,Hin,S] in fp8 so per-core DMAs are contiguous
    enc_t = np.ascontiguousarray(np.transpose(enc, (1, 2, 0)).astype(F8))
    hid_t = np.asarray(hidden, dtype=np.float32).T.astype(F8)  # [H=k, B]
    maskneg = np.where(np.asarray(mask, dtype=bool), np.float32(-1e30),
                       np.float32(0.0)).astype(np.float32)
    W1 = np.asarray(W1, dtype=np.float32)
    w1e8 = (WS * W1[:, :HIN].T).astype(F8)   # [K=HIN, H]
    w1h8 = (WS * W1[:, HIN:].T).astype(F8)   # [K=H, H]
    # w1e packed [ht, p, it, m] = w1e8[it*128+p, ht*128+m]
    w1e_pack = np.ascontiguousarray(
        w1e8.reshape(IT, P, HT, P).transpose(2, 1, 0, 3))
    # w1h packed [p, it*H+h] = w1h8[it*128+p, h]
    w1h_pack = np.ascontiguousarray(
        w1h8.reshape(IT, P, H).transpose(1, 0, 2).reshape(P, IT * H))
    b1 = np.ascontiguousarray(np.asarray(b1, dtype=np.float32).reshape(H))
    # w2 packed [p, ht] = W2[ht*128+p], f32 per-partition scalars
    w2_pack = np.ascontiguousarray(
        np.asarray(W2, dtype=np.float32).reshape(HT, P).T)

    in_maps = []
    for c in range(N_CORES):
        sl = slice(c * BL, (c + 1) * BL)
        # hidden packed [p, it, b] = hid_t[it*128+p, b]
        hid_pack = np.ascontiguousarray(
            hid_t[:, sl].reshape(IT, P, BL).transpose(1, 0, 2))
        in_maps.append({
            "enc": np.ascontiguousarray(enc_t[sl]),
            "hiddent": hid_pack,
            "maskneg": np.ascontiguousarray(maskneg[sl].reshape(-1)),
            "w1e": w1e_pack,
            "w1h": w1h_pack,
            "b1": b1,
            "w2": w2_pack,
        })

    res = run_bass_kernel_spmd(nc, in_maps, core_ids=list(range(N_CORES)))
    LAST_RESULT = res
    out = np.concatenate([res.results[c]["out"] for c in range(N_CORES)], axis=0)
    return np.ascontiguousarray(out[:, None, :].astype(np.float32))


# revision 28
# speedup vs baseline: 1.9958x; 1.2991x over previous
"""Trainium2 Bass kernel for the attention-scoring MLP (nn_Attn):

    enc = encoder_outputs.transpose(1,0,2)          # [B,S,Hin]
    a1  = tanh(enc @ W1_enc.T + hidden @ W1_hid.T + b1)
    s   = a1 @ W2[0] (+ b2 -- dropped: softmax shift-invariant)
    s   = where(mask, -inf, s)
    out = softmax(s, axis=-1)[:, None, :]           # [B,1,S]

Sharding: data-parallel over batch B=32 across 8 NeuronCores (4 rows
each), weights replicated, no collectives.

Mask packing: masked positions get score -inf and contribute nothing to
the softmax, so the kernel only computes the ~50% unmasked columns.
kernel() gathers each row's unmasked enc columns into a packed layout of
CB columns per row (CB = 576 covers the binomial spread; zero columns +
-1e30 mask data pad the remainder, so the instruction stream is
SPMD-uniform across cores -- only DMA'd data differs). The host scatters
the packed attention weights back to full [B,1,S] (pure layout, no
arithmetic).

Per core the main matmul is computed transposed -- a1T[h, s] =
W1_encT.T @ encT -- so the (b1 + hidden@W1_hid.T) term rides the
ScalarEngine's per-partition bias port of the tanh activation. Main
matmuls run in fp8 (e4m3) with MatmulPerfMode.DoubleRow: each matmul
contracts a PAIR of 128-row k-tiles (lhsT/rhs laid out [128, 2, n]), the
PE streaming 2 fp8 rows/cycle. Weights are pre-scaled by 32 host-side so
their +-1/sqrt(2048) range sits in fp8's normal range; the 1/32 is
folded into the tanh activation scale. Accumulation is fp32 in PSUM.

The w2 score contraction runs as a DVE accumulate chain over ht
(acc = w2[:,ht]*th_ht + acc, per-partition scalars) plus a single bf16
ones-matmul to reduce partitions; the final tile instead issues M=1
score matmuls on the PE so the tail never waits on the DVE chain.
"""

import numpy as np
import ml_dtypes

import concourse.bass as bass
import concourse.tile as tile
from concourse import bacc, mybir
from concourse.bass import ds, ts
from concourse.bass_utils import run_bass_kernel_spmd
from concourse.masks import make_identity

N_CORES = 8
B, S, HIN, H = 32, 1024, 1024, 1024
BL = B // N_CORES          # local batch rows per core
P = 128                    # partitions
IT = HIN // P              # contraction k-tiles
KP = IT // 2               # k-tile pairs (DoubleRow)
HT = H // P                # output-feature tiles
NT = 512                   # moving-dim tile (s columns per matmul)
WS = 32.0                  # host-side weight scale (undone in act scale)
F32 = mybir.dt.float32
BF16 = mybir.dt.bfloat16
FP8 = mybir.dt.float8e4
AF = mybir.ActivationFunctionType
DR = mybir.MatmulPerfMode.DoubleRow
F8 = ml_dtypes.float8_e4m3

_cached = {}
LAST_RESULT = None  # BassKernelResults of the most recent run (for test harness)


def _layout(CB):
    """Static tile/segment layout for packed width CB (multiple of 64)."""
    TCOLS = BL * CB
    tiles = []  # (col0, nt, [(off, ln, b), ...])
    col = 0
    while col < TCOLS:
        nt = min(NT, TCOLS - col)
        segs = []
        o = col
        while o < col + nt:
            b = o // CB
            end = min((b + 1) * CB, col + nt)
            segs.append((o - col, end - o, b))
            o = end
        tiles.append((col, nt, segs))
        col += nt
    slot_of = {}
    b_slots = {b: [] for b in range(BL)}
    s = 0
    for ti, (_, _, segs) in enumerate(tiles):
        for si, (_, _, b) in enumerate(segs):
            slot_of[(ti, si)] = s
            b_slots[b].append(s)
            s += 1
    return TCOLS, tiles, slot_of, b_slots, s


def _build(CB):
    if CB in _cached:
        return _cached[CB]
    TCOLS, tiles, slot_of, b_slots, NSEG = _layout(CB)
    NTI = len(tiles)

    nc = bacc.Bacc("TRN2", target_bir_lowering=False, debug=False,
                   num_devices=N_CORES)

    # packed encT: [k, col] fp8 (col = b*CB + packed s)
    enc_ext = nc.dram_tensor("enc", [HIN, TCOLS], FP8, kind="ExternalInput").ap()
    # hiddenT packed [p, it, b] fp8 (k = it*128+p)
    hidt_ext = nc.dram_tensor("hiddent", [P, IT, BL], FP8, kind="ExternalInput").ap()
    mneg_ext = nc.dram_tensor("maskneg", [TCOLS], F32, kind="ExternalInput").ap()
    # W1_enc.T packed per ht: [ht, p, it, m] = 32*W1[ht*128+m, it*128+p]
    w1e_ext = nc.dram_tensor("w1e", [HT, P, IT, P], FP8, kind="ExternalInput").ap()
    # W1_hid.T packed: [p, it*H + h]
    w1h_ext = nc.dram_tensor("w1h", [P, IT * H], FP8, kind="ExternalInput").ap()
    b1_ext = nc.dram_tensor("b1", [H], F32, kind="ExternalInput").ap()
    # W2 packed [p, ht] f32 (h = ht*128+p)
    w2_ext = nc.dram_tensor("w2", [P, HT], F32, kind="ExternalInput").ap()
    out_ext = nc.dram_tensor("out", [TCOLS], F32, kind="ExternalOutput").ap()

    with tile.TileContext(nc) as tc:
        with (
            tc.tile_pool(name="consts", bufs=1) as consts,
            tc.tile_pool(name="encp", bufs=3) as encp,
            tc.tile_pool(name="thp", bufs=5) as thp,
            tc.tile_pool(name="accp", bufs=3) as accp,
            tc.tile_pool(name="pap", bufs=2, space="PSUM") as pap,
            tc.tile_pool(name="pscp", bufs=2, space="PSUM") as pscp,
            tc.tile_pool(name="psA", bufs=1, space="PSUM") as psA,
            tc.tile_pool(name="psT", bufs=2, space="PSUM") as psTp,
        ):
            # ---- PE warmup: junk matmuls with no DMA deps so the HAM
            # clock-gate is ramping toward 8/8 when the real matmuls arrive.
            warm_sb = consts.tile([P, NT], BF16)
            nc.gpsimd.memset(warm_sb[:], 0.0)
            warm_ps = pap.tile([P, NT], F32, tag="pa1")
            for _ in range(10):
                nc.tensor.matmul(warm_ps[:], warm_sb[:, 0:P], warm_sb[:],
                                 start=True, stop=True)

            # ---- resident weights/constants ----
            # DMA emission order = ring service order: first-needed first.
            hT_sb = consts.tile([P, IT, BL], FP8)
            nc.sync.dma_start(hT_sb[:], hidt_ext[:])
            w1e_t = []
            for ht in range(HT):
                w = consts.tile([P, IT, P], FP8, tag=f"w1e{ht}")
                nc.sync.dma_start(w[:], w1e_ext[ht])
                w1e_t.append(w)
            b1T_sb = consts.tile([P, HT], F32)
            nc.sync.dma_start(b1T_sb[:], b1_ext.rearrange("(ht p) -> p ht", p=P))
            w2T_sb = consts.tile([P, HT], F32)
            nc.sync.dma_start(w2T_sb[:], w2_ext[:])
            mneg_sb = consts.tile([1, TCOLS], F32)
            nc.sync.dma_start(mneg_sb[:], mneg_ext[:])
            # scalar ring: w1h (needed by phase A, overlaps warmup) then the
            # first enc tile.
            w1h_sb = consts.tile([P, IT, H], FP8)
            for it in range(IT):
                nc.scalar.dma_start(w1h_sb[:, it, :], w1h_ext[:, ds(it * H, H)])
            enc0_sb = encp.tile([P, IT, NT], FP8, tag="enc", name="enc")
            for it in range(IT):
                nc.scalar.dma_start(enc0_sb[:, it, 0:tiles[0][1]],
                                    enc_ext[ds(it * P, P), ds(0, tiles[0][1])])
            ident_sb = consts.tile([BL, BL], F32)
            make_identity(nc, ident_sb[:])
            ones_sb = consts.tile([P, 1], BF16)
            nc.gpsimd.memset(ones_sb[:], 1.0)
            w2b_sb = consts.tile([P, HT], BF16)
            nc.vector.tensor_copy(w2b_sb[:], w2T_sb[:])

            bias_sb = consts.tile([P, HT * BL], F32)   # [p, ht*BL+b]
            hterm_sb = consts.tile([BL, H], F32)
            scores_sb = consts.tile([1, TCOLS], F32)
            c40 = consts.tile([1, 1], F32)
            nc.gpsimd.memset(c40[:], -40.0)
            exps = consts.tile([1, TCOLS], F32)
            ssum = consts.tile([1, NSEG], F32)
            rcp = consts.tile([1, BL], F32)
            attn = consts.tile([1, TCOLS], F32)

            # ---- phase A: h_term[b,h] = hidden @ W1_hid.T (x32 in fp8);
            # bias = h_termT/32 + b1T. Plain fp8 matmuls (no DoubleRow: the
            # dual-fp8 ldweights ISA check needs pair-dim step%16==0, and
            # these stationaries are tiny anyway).
            pht = psA.tile([BL, H], F32)
            for g in range(2):
                for it in range(IT):
                    nc.tensor.matmul(pht[:, ds(g * NT, NT)],
                                     hT_sb[:, it, :],
                                     w1h_sb[:, it, ds(g * NT, NT)],
                                     start=(it == 0), stop=(it == IT - 1))
            nc.scalar.mul(hterm_sb[:], pht[:], 1.0 / WS)
            for ht in range(HT):
                ptT = psTp.tile([P, BL], F32)
                nc.tensor.transpose(ptT[:], hterm_sb[:, ts(ht, P)], ident_sb[:])
                nc.vector.tensor_scalar_add(bias_sb[:, ts(ht, BL)], ptT[:],
                                            b1T_sb[:, ds(ht, 1)])

            # ---- phase B ----
            def flush_scores(ti, acc_f, psc=None):
                c0, nt, segs = tiles[ti]
                if psc is None:
                    psc = pscp.tile([1, NT], F32, tag="psc", name="psc")
                    nc.tensor.matmul(psc[0:1, 0:nt], ones_sb[:],
                                     acc_f[:, 0:nt], start=True, stop=True)
                # scores += mask * -1e30 (also kills the packing pad columns)
                nc.vector.tensor_add(scores_sb[0:1, ds(c0, nt)],
                                     psc[0:1, 0:nt],
                                     mneg_sb[0:1, ds(c0, nt)])
                # |scores| <= ||W2||_1 <= 32: exp(s - 40) never overflows and
                # softmax is shift-invariant -- no max-reduce needed.
                for si, (off, ln, b) in enumerate(segs):
                    slot = slot_of[(ti, si)]
                    nc.scalar.activation(exps[0:1, ds(c0 + off, ln)],
                                         scores_sb[0:1, ds(c0 + off, ln)],
                                         AF.Exp, bias=c40[0:1, 0:1], scale=1.0,
                                         accum_out=ssum[0:1, ds(slot, 1)])
                    if slot == b_slots[b][-1]:
                        s0 = b_slots[b][0]
                        nsl = len(b_slots[b])
                        nc.vector.reduce_sum(rcp[0:1, ds(b, 1)],
                                             ssum[0:1, ds(s0, nsl)],
                                             axis=mybir.AxisListType.X)
                        nc.vector.reciprocal(rcp[0:1, ds(b, 1)],
                                             rcp[0:1, ds(b, 1)])
                        nc.vector.tensor_scalar_mul(attn[0:1, ds(b * CB, CB)],
                                                    exps[0:1, ds(b * CB, CB)],
                                                    rcp[0:1, ds(b, 1)])
                        nc.sync.dma_start(out_ext[ds(b * CB, CB)],
                                          attn[0:1, ds(b * CB, CB)])

            prev = None  # (tile idx, final acc tile) awaiting score flush
            for ti, (c0, nt, segs) in enumerate(tiles):
                last = ti == NTI - 1
                if ti == 0:
                    enc_sb = enc0_sb
                else:
                    enc_sb = encp.tile([P, IT, NT], FP8, tag="enc", name="enc")
                    # ti==1 rides the scalar ring (startup overlap with w1e on
                    # sync); steady-state tiles use the otherwise-idle sync
                    # ring so DMA triggers never serialize against tanh on ACT.
                    eng = nc.scalar if ti == 1 else nc.sync
                    for it in range(IT):
                        eng.dma_start(
                            enc_sb[:, it, 0:nt],
                            enc_ext[ds(it * P, P), ds(c0, nt)],
                        )
                acc = None
                psc_last = None
                pend_sc = []
                for ht in range(HT):
                    pa1 = pap.tile([P, NT], F32, tag="pa1")
                    for k in range(KP):
                        nc.tensor.matmul(
                            pa1[:, 0:nt],
                            w1e_t[ht][:, ds(2 * k, 2), :],
                            enc_sb[:, ds(2 * k, 2), 0:nt],
                            start=(k == 0), stop=(k == KP - 1),
                            perf_mode=DR,
                        )
                    # On the last tile, drain pending PE score matmuls two
                    # groups behind the tanh that feeds them.
                    if len(pend_sc) > 2:
                        pht_, pth_ = pend_sc.pop(0)
                        nc.tensor.matmul(psc_last[0:1, 0:nt],
                                         w2b_sb[:, ds(pht_, 1)],
                                         pth_[:, 0:nt], start=(pht_ == 0),
                                         stop=(pht_ == HT - 1))
                    th = thp.tile([P, NT], BF16, tag="th", name="th")
                    for off, ln, b in segs:
                        nc.scalar.activation(th[:, ds(off, ln)],
                                             pa1[:, ds(off, ln)], AF.Tanh,
                                             bias=bias_sb[:, ds(ht * BL + b, 1)],
                                             scale=1.0 / WS)
                    if last:
                        # last tile: w2 contraction on the PE (plain bf16,
                        # M=1) so the tail never waits on the DVE chain.
                        if ht == 0:
                            psc_last = pscp.tile([1, NT], F32, tag="psc",
                                                 name="psc")
                        pend_sc.append((ht, th))
                    else:
                        nxt = accp.tile([P, NT], BF16, tag="acc", name="acc")
                        if ht == 0:
                            nc.vector.tensor_scalar_mul(nxt[:, 0:nt],
                                                        th[:, 0:nt],
                                                        w2T_sb[:, ds(0, 1)])
                        else:
                            nc.vector.scalar_tensor_tensor(
                                nxt[:, 0:nt], th[:, 0:nt],
                                w2T_sb[:, ds(ht, 1)], acc[:, 0:nt],
                                mybir.AluOpType.mult, mybir.AluOpType.add)
                        acc = nxt
                    # Flush the previous tile's scores once this tile's PE
                    # pipeline is 3 groups deep (never stalls the in-order
                    # PE).
                    if ht == 2 and prev is not None:
                        flush_scores(*prev)
                        prev = None
                if last:
                    for pht_, pth_ in pend_sc:
                        nc.tensor.matmul(psc_last[0:1, 0:nt],
                                         w2b_sb[:, ds(pht_, 1)],
                                         pth_[:, 0:nt], start=(pht_ == 0),
                                         stop=(pht_ == HT - 1))
                    if prev is not None:
                        flush_scores(*prev)
                        prev = None
                    flush_scores(ti, None, psc=psc_last)
                else:
                    prev = (ti, acc)

    nc.compile()
    _cached[CB] = (nc, TCOLS, tiles)
    return _cached[CB]


def kernel(hidden, encoder_outputs, mask, W1, b1, W2, b2):
    global LAST_RESULT

    mask = np.asarray(mask, dtype=bool)
    idx_all = [np.nonzero(~mask[gb])[0] for gb in range(B)]
    maxcnt = max(len(ix) for ix in idx_all)
    CB = max(576, -(-maxcnt // 64) * 64)
    nc, TCOLS, _ = _build(CB)

    enc = np.asarray(encoder_outputs, dtype=np.float32)
    # [S,B,Hin] -> [B,Hin,S] in fp8 so per-core DMAs are contiguous
    enc_t = np.ascontiguousarray(np.transpose(enc, (1, 2, 0)).astype(F8))
    hid_t = np.asarray(hidden, dtype=np.float32).T.astype(F8)  # [H=k, B]
    W1 = np.asarray(W1, dtype=np.float32)
    w1e8 = (WS * W1[:, :HIN].T).astype(F8)   # [K=HIN, H]
    w1h8 = (WS * W1[:, HIN:].T).astype(F8)   # [K=H, H]
    # w1e packed [ht, p, it, m] = w1e8[it*128+p, ht*128+m]
    w1e_pack = np.ascontiguousarray(
        w1e8.reshape(IT, P, HT, P).transpose(2, 1, 0, 3))
    # w1h packed [p, it*H+h] = w1h8[it*128+p, h]
    w1h_pack = np.ascontiguousarray(
        w1h8.reshape(IT, P, H).transpose(1, 0, 2).reshape(P, IT * H))
    b1 = np.ascontiguousarray(np.asarray(b1, dtype=np.float32).reshape(H))
    # w2 packed [p, ht] = W2[ht*128+p], f32 per-partition scalars
    w2_pack = np.ascontiguousarray(
        np.asarray(W2, dtype=np.float32).reshape(HT, P).T)

    in_maps = []
    for c in range(N_CORES):
        sl = slice(c * BL, (c + 1) * BL)
        hid_pack = np.ascontiguousarray(
            hid_t[:, sl].reshape(IT, P, BL).transpose(1, 0, 2))
        enc_pack = np.zeros((HIN, TCOLS), dtype=F8)
        mneg = np.full(TCOLS, -1e30, dtype=np.float32)
        for b in range(BL):
            ix = idx_all[c * BL + b]
            enc_pack[:, b * CB:b * CB + len(ix)] = enc_t[c * BL + b][:, ix]
            mneg[b * CB:b * CB + len(ix)] = 0.0
        in_maps.append({
            "enc": enc_pack,
            "hiddent": hid_pack,
            "maskneg": mneg,
            "w1e": w1e_pack,
            "w1h": w1h_pack,
            "b1": b1,
            "w2": w2_pack,
        })

    res = run_bass_kernel_spmd(nc, in_maps, core_ids=list(range(N_CORES)))
    LAST_RESULT = res
    out = np.zeros((B, S), dtype=np.float32)
    for c in range(N_CORES):
        packed = res.results[c]["out"]
        for b in range(BL):
            gb = c * BL + b
            ix = idx_all[gb]
            out[gb, ix] = packed[b * CB:b * CB + len(ix)]
    return np.ascontiguousarray(out[:, None, :])


# revision 30
# speedup vs baseline: 2.5114x; 1.2584x over previous
"""Trainium2 Bass kernel for the attention-scoring MLP (nn_Attn):

    enc = encoder_outputs.transpose(1,0,2)          # [B,S,Hin]
    a1  = tanh(enc @ W1_enc.T + hidden @ W1_hid.T + b1)
    s   = a1 @ W2[0] (+ b2 -- dropped: softmax shift-invariant)
    s   = where(mask, -inf, s)
    out = softmax(s, axis=-1)[:, None, :]           # [B,1,S]

Sharding: data-parallel over batch B=32 across 8 NeuronCores (4 rows
each), weights replicated, no collectives.

Mask packing: masked positions get score -inf and contribute nothing to
the softmax, so the kernel only computes the ~50% unmasked columns.
kernel() gathers each row's unmasked enc columns into a packed layout of
CB columns per row (CB = 576 covers the binomial spread; zero columns +
-1e30 mask data pad the remainder, so the instruction stream is
SPMD-uniform across cores -- only DMA'd data differs). The host scatters
the packed attention weights back to full [B,1,S] (pure layout, no
arithmetic).

Per core the main matmul is computed transposed -- a1T[h, s] =
W1_encT.T @ encT -- so the (b1 + hidden@W1_hid.T) term rides the
ScalarEngine's per-partition bias port of the tanh activation. Matmuls
run in fp8 (e4m3) with MatmulPerfMode.DoubleRow: each matmul contracts a
PAIR of 128-row k-tiles (lhsT/rhs laid out [128, 2, n]), the PE
streaming 2 fp8 rows/cycle. Weights are pre-scaled by 32 host-side so
their +-1/sqrt(2048) range sits in fp8's normal range; the 1/32 is
folded into the tanh activation scale. Accumulation is fp32 in PSUM.
Each tile's enc arrives in ONE DMA instruction (descriptor generation at
~600ns/instruction was throttling the pipeline when split per k-tile).

The w2 score contraction runs as an in-place DVE accumulate chain over
ht (acc = w2[:,ht]*th_ht + acc, per-partition scalars) plus a single
bf16 ones-matmul to reduce partitions; the final tile instead issues M=1
score matmuls on the PE so the tail never waits on the DVE chain.
"""

import numpy as np
import ml_dtypes

import concourse.bass as bass
import concourse.tile as tile
from concourse import bacc, mybir
from concourse.bass import ds, ts
from concourse.bass_utils import run_bass_kernel_spmd
from concourse.masks import make_identity

N_CORES = 8
B, S, HIN, H = 32, 1024, 1024, 1024
BL = B // N_CORES          # local batch rows per core
P = 128                    # partitions
IT = HIN // P              # contraction k-tiles
KP = IT // 2               # k-tile pairs (DoubleRow)
HT = H // P                # output-feature tiles
NT = 512                   # moving-dim tile (s columns per matmul)
BP = 16                    # padded batch rows (dual-fp8 ldweights step%16)
WS = 32.0                  # host-side weight scale (undone in act scale)
F32 = mybir.dt.float32
BF16 = mybir.dt.bfloat16
FP8 = mybir.dt.float8e4
AF = mybir.ActivationFunctionType
DR = mybir.MatmulPerfMode.DoubleRow
F8 = ml_dtypes.float8_e4m3

_cached = {}
LAST_RESULT = None  # BassKernelResults of the most recent run (for test harness)


def _layout(CB):
    """Static tile/segment layout for packed width CB (multiple of 64)."""
    TCOLS = BL * CB
    tiles = []  # (col0, nt, [(off, ln, b), ...])
    col = 0
    while col < TCOLS:
        nt = min(NT, TCOLS - col)
        segs = []
        o = col
        while o < col + nt:
            b = o // CB
            end = min((b + 1) * CB, col + nt)
            segs.append((o - col, end - o, b))
            o = end
        tiles.append((col, nt, segs))
        col += nt
    slot_of = {}
    b_slots = {b: [] for b in range(BL)}
    s = 0
    for ti, (_, _, segs) in enumerate(tiles):
        for si, (_, _, b) in enumerate(segs):
            slot_of[(ti, si)] = s
            b_slots[b].append(s)
            s += 1
    return TCOLS, tiles, slot_of, b_slots, s


def _build(CB):
    if CB in _cached:
        return _cached[CB]
    TCOLS, tiles, slot_of, b_slots, NSEG = _layout(CB)
    NTI = len(tiles)

    nc = bacc.Bacc("TRN2", target_bir_lowering=False, debug=False,
                   num_devices=N_CORES)

    # packed encT: [it, p, col] fp8 (k = it*128+p; col = b*CB + packed s)
    enc_ext = nc.dram_tensor("enc", [IT, P, TCOLS], FP8, kind="ExternalInput").ap()
    # hiddenT packed+padded [p, it, bp] fp8 (k = it*128+p; bp 0..3 real)
    hidt_ext = nc.dram_tensor("hiddent", [P, IT, BP], FP8, kind="ExternalInput").ap()
    mneg_ext = nc.dram_tensor("maskneg", [TCOLS], F32, kind="ExternalInput").ap()
    # W1_enc.T packed per ht: [ht, p, it, m] = 32*W1[ht*128+m, it*128+p]
    w1e_ext = nc.dram_tensor("w1e", [HT, P, IT, P], FP8, kind="ExternalInput").ap()
    # W1_hid.T packed: [p, it, h]
    w1h_ext = nc.dram_tensor("w1h", [P, IT, H], FP8, kind="ExternalInput").ap()
    b1_ext = nc.dram_tensor("b1", [H], F32, kind="ExternalInput").ap()
    # W2 packed [p, ht] f32 (h = ht*128+p)
    w2_ext = nc.dram_tensor("w2", [P, HT], F32, kind="ExternalInput").ap()
    out_ext = nc.dram_tensor("out", [TCOLS], F32, kind="ExternalOutput").ap()

    encR = enc_ext.rearrange("it p c -> p it c")

    with tile.TileContext(nc) as tc:
        with (
            tc.tile_pool(name="consts", bufs=1) as consts,
            tc.tile_pool(name="encp", bufs=3) as encp,
            tc.tile_pool(name="thp", bufs=2) as thp,
            tc.tile_pool(name="accp", bufs=2) as accp,
            tc.tile_pool(name="pap", bufs=4, space="PSUM") as pap,
            tc.tile_pool(name="pscp", bufs=2, space="PSUM") as pscp,
            tc.tile_pool(name="psA", bufs=1, space="PSUM") as psA,
            tc.tile_pool(name="psT", bufs=1, space="PSUM") as psTp,
        ):
            # ---- resident weights/constants; DMA emission order = ring
            # service order: first-needed first. sync: w1e0, hT, w1e1-7,
            # small consts. scalar: enc tile0, w1h, enc tile1.
            w1e_t = []
            w = consts.tile([P, IT, P], FP8, tag="w1e0", name="w1e0")
            nc.sync.dma_start(w[:], w1e_ext[0])
            w1e_t.append(w)
            hT_sb = consts.tile([P, IT, BP], FP8)
            nc.sync.dma_start(hT_sb[:], hidt_ext[:])
            for ht in range(1, HT):
                w = consts.tile([P, IT, P], FP8, tag=f"w1e{ht}", name=f"w1e{ht}")
                nc.sync.dma_start(w[:], w1e_ext[ht])
                w1e_t.append(w)
            b1T_sb = consts.tile([P, HT], F32)
            nc.sync.dma_start(b1T_sb[:], b1_ext.rearrange("(ht p) -> p ht", p=P))
            w2T_sb = consts.tile([P, HT], F32)
            nc.sync.dma_start(w2T_sb[:], w2_ext[:])
            mneg_sb = consts.tile([1, TCOLS], F32)
            nc.sync.dma_start(mneg_sb[:], mneg_ext[:])
            enc0_sb = encp.tile([P, IT, NT], FP8, tag="enc", name="enc")
            nc.scalar.dma_start(enc0_sb[:, :, 0:tiles[0][1]],
                                encR[:, :, ds(0, tiles[0][1])])
            w1h_sb = consts.tile([P, IT, H], FP8)
            nc.scalar.dma_start(w1h_sb[:], w1h_ext[:])

            # ---- PE warmup: junk matmuls with no DMA deps so the HAM
            # clock-gate / p-state ramp toward full speed during the preamble.
            warm_sb = consts.tile([P, NT], BF16)
            nc.gpsimd.memset(warm_sb[:], 0.0)
            warm_ps = pap.tile([P, NT], F32, tag="pa1")
            for _ in range(10):
                nc.tensor.matmul(warm_ps[:], warm_sb[:, 0:P], warm_sb[:],
                                 start=True, stop=True)

            ident_sb = consts.tile([BL, BL], F32)
            make_identity(nc, ident_sb[:])
            ones_sb = consts.tile([P, 1], BF16)
            nc.gpsimd.memset(ones_sb[:], 1.0)
            w2b_sb = consts.tile([P, HT], BF16)
            nc.vector.tensor_copy(w2b_sb[:], w2T_sb[:])

            bias_sb = consts.tile([P, HT * BL], F32)   # [p, ht*BL+b]
            hterm_sb = consts.tile([BL, H], F32)
            scores_sb = consts.tile([1, TCOLS], F32)
            c40 = consts.tile([1, 1], F32)
            nc.gpsimd.memset(c40[:], -40.0)
            exps = consts.tile([1, TCOLS], F32)
            ssum = consts.tile([1, NSEG], F32)
            rcp = consts.tile([1, BL], F32)
            attn = consts.tile([1, TCOLS], F32)

            def phase_a():
                # h_term[b,h] = hidden @ W1_hid.T (x32 in fp8, DoubleRow with
                # the batch dim padded to 16 so the dual-fp8 ldweights pair
                # step is 16); bias = h_termT/32 + b1T.
                ptT = psTp.tile([P, HT * BL], F32)
                for g in range(2):
                    pht = psA.tile([BP, NT], F32, tag="pht", name="pht")
                    for k in range(KP):
                        nc.tensor.matmul(pht[:],
                                         hT_sb[:, ds(2 * k, 2), :],
                                         w1h_sb[:, ds(2 * k, 2), ds(g * NT, NT)],
                                         start=(k == 0), stop=(k == KP - 1),
                                         perf_mode=DR)
                    nc.scalar.mul(hterm_sb[:, ds(g * NT, NT)], pht[0:BL, :],
                                  1.0 / WS)
                for ht in range(HT):
                    nc.tensor.transpose(ptT[:, ts(ht, BL)],
                                        hterm_sb[:, ts(ht, P)], ident_sb[:])
                    nc.vector.tensor_scalar_add(bias_sb[:, ts(ht, BL)],
                                                ptT[:, ts(ht, BL)],
                                                b1T_sb[:, ds(ht, 1)])

            # ---- phase B ----
            def flush_scores(ti, acc_f, psc=None):
                c0, nt, segs = tiles[ti]
                if psc is None:
                    psc = pscp.tile([1, NT], F32, tag="psc", name="psc")
                    nc.tensor.matmul(psc[0:1, 0:nt], ones_sb[:],
                                     acc_f[:, 0:nt], start=True, stop=True)
                # scores += mask * -1e30 (also kills the packing pad columns)
                nc.vector.tensor_add(scores_sb[0:1, ds(c0, nt)],
                                     psc[0:1, 0:nt],
                                     mneg_sb[0:1, ds(c0, nt)])
                # |scores| <= ||W2||_1 <= 32: exp(s - 40) never overflows and
                # softmax is shift-invariant -- no max-reduce needed.
                for si, (off, ln, b) in enumerate(segs):
                    slot = slot_of[(ti, si)]
                    nc.scalar.activation(exps[0:1, ds(c0 + off, ln)],
                                         scores_sb[0:1, ds(c0 + off, ln)],
                                         AF.Exp, bias=c40[0:1, 0:1], scale=1.0)
                    nc.vector.reduce_sum(ssum[0:1, ds(slot, 1)],
                                         exps[0:1, ds(c0 + off, ln)],
                                         axis=mybir.AxisListType.X)
                    if slot == b_slots[b][-1]:
                        s0 = b_slots[b][0]
                        nsl = len(b_slots[b])
                        nc.vector.reduce_sum(rcp[0:1, ds(b, 1)],
                                             ssum[0:1, ds(s0, nsl)],
                                             axis=mybir.AxisListType.X)
                        nc.vector.reciprocal(rcp[0:1, ds(b, 1)],
                                             rcp[0:1, ds(b, 1)])
                        nc.vector.tensor_scalar_mul(attn[0:1, ds(b * CB, CB)],
                                                    exps[0:1, ds(b * CB, CB)],
                                                    rcp[0:1, ds(b, 1)])
                        nc.sync.dma_start(out_ext[ds(b * CB, CB)],
                                          attn[0:1, ds(b * CB, CB)])

            prev = None  # (tile idx, final acc tile) awaiting score flush
            for ti, (c0, nt, segs) in enumerate(tiles):
                last = ti == NTI - 1
                if ti == 0:
                    enc_sb = enc0_sb
                else:
                    enc_sb = encp.tile([P, IT, NT], FP8, tag="enc", name="enc")
                    eng = nc.scalar if ti == 1 else nc.sync
                    eng.dma_start(enc_sb[:, :, 0:nt], encR[:, :, ds(c0, nt)])
                acc = None
                psc_last = None
                pend_sc = []
                th_big = thp.tile([P, HT, NT], BF16, tag="th", name="th")

                def main_group(ht):
                    pa1 = pap.tile([P, NT], F32, tag="pa1", name="pa1")
                    for k in range(KP):
                        nc.tensor.matmul(
                            pa1[:, 0:nt],
                            w1e_t[ht][:, ds(2 * k, 2), :],
                            enc_sb[:, ds(2 * k, 2), 0:nt],
                            start=(k == 0), stop=(k == KP - 1),
                            perf_mode=DR,
                        )
                    return pa1

                # tile0 runs its first 4 PSUM groups before phase A so the PE
                # works on (early-arriving) enc while w1h is still in flight;
                # phase A's ACT/PE ops are emitted before any tanh, keeping
                # both in-order queues deadlock-free (tanh needs phase A's
                # bias).
                pa1_pend = {}
                if ti == 0:
                    for ht in range(4):
                        pa1_pend[ht] = main_group(ht)
                    phase_a()
                for ht in range(HT):
                    pa1 = pa1_pend.pop(ht, None)
                    if pa1 is None:
                        pa1 = main_group(ht)
                    # On the last tile, drain pending PE score matmuls two
                    # groups behind the tanh that feeds them.
                    if len(pend_sc) > 2:
                        pht_ = pend_sc.pop(0)
                        nc.tensor.matmul(psc_last[0:1, 0:nt],
                                         w2b_sb[:, ds(pht_, 1)],
                                         th_big[:, pht_, 0:nt],
                                         start=(pht_ == 0),
                                         stop=(pht_ == HT - 1))
                    for off, ln, b in segs:
                        nc.scalar.activation(th_big[:, ht, ds(off, ln)],
                                             pa1[:, ds(off, ln)], AF.Tanh,
                                             bias=bias_sb[:, ds(ht * BL + b, 1)],
                                             scale=1.0 / WS)
                    if last:
                        # last tile: w2 contraction on the PE (plain bf16,
                        # M=1) so the tail never waits on the DVE chain.
                        if ht == 0:
                            psc_last = pscp.tile([1, NT], F32, tag="psc",
                                                 name="psc")
                        pend_sc.append(ht)
                    elif ht == 0:
                        acc = accp.tile([P, NT], BF16, tag="acc", name="acc")
                        nc.vector.tensor_scalar_mul(acc[:, 0:nt],
                                                    th_big[:, 0, 0:nt],
                                                    w2T_sb[:, ds(0, 1)])
                    else:
                        nc.vector.scalar_tensor_tensor(
                            acc[:, 0:nt], th_big[:, ht, 0:nt],
                            w2T_sb[:, ds(ht, 1)], acc[:, 0:nt],
                            mybir.AluOpType.mult, mybir.AluOpType.add)
                    # Flush the previous tile's scores once this tile's PE
                    # pipeline is deep enough (never stalls the in-order PE).
                    if ht == 2 and prev is not None:
                        flush_scores(*prev)
                        prev = None
                if last:
                    for pht_ in pend_sc:
                        nc.tensor.matmul(psc_last[0:1, 0:nt],
                                         w2b_sb[:, ds(pht_, 1)],
                                         th_big[:, pht_, 0:nt],
                                         start=(pht_ == 0),
                                         stop=(pht_ == HT - 1))
                    if prev is not None:
                        flush_scores(*prev)
                        prev = None
                    flush_scores(ti, None, psc=psc_last)
                else:
                    prev = (ti, acc)

    nc.compile()
    _cached[CB] = (nc, TCOLS, tiles)
    return _cached[CB]


def kernel(hidden, encoder_outputs, mask, W1, b1, W2, b2):
    global LAST_RESULT

    mask = np.asarray(mask, dtype=bool)
    idx_all = [np.nonzero(~mask[gb])[0] for gb in range(B)]
    maxcnt = max(len(ix) for ix in idx_all)
    CB = max(576, -(-maxcnt // 64) * 64)
    nc, TCOLS, _ = _build(CB)

    enc = np.asarray(encoder_outputs, dtype=np.float32)
    # [S,B,Hin] -> [B,Hin,S] in fp8 so per-core DMAs are contiguous
    enc_t = np.ascontiguousarray(np.transpose(enc, (1, 2, 0)).astype(F8))
    hid_t = np.asarray(hidden, dtype=np.float32).T.astype(F8)  # [H=k, B]
    W1 = np.asarray(W1, dtype=np.float32)
    w1e8 = (WS * W1[:, :HIN].T).astype(F8)   # [K=HIN, H]
    w1h8 = (WS * W1[:, HIN:].T).astype(F8)   # [K=H, H]
    # w1e packed [ht, p, it, m] = w1e8[it*128+p, ht*128+m]
    w1e_pack = np.ascontiguousarray(
        w1e8.reshape(IT, P, HT, P).transpose(2, 1, 0, 3))
    # w1h packed [p, it, h] = w1h8[it*128+p, h]
    w1h_pack = np.ascontiguousarray(
        w1h8.reshape(IT, P, H).transpose(1, 0, 2))
    b1 = np.ascontiguousarray(np.asarray(b1, dtype=np.float32).reshape(H))
    # w2 packed [p, ht] = W2[ht*128+p], f32 per-partition scalars
    w2_pack = np.ascontiguousarray(
        np.asarray(W2, dtype=np.float32).reshape(HT, P).T)

    in_maps = []
    for c in range(N_CORES):
        sl = slice(c * BL, (c + 1) * BL)
        hid_pack = np.zeros((P, IT, BP), dtype=F8)
        hid_pack[:, :, 0:BL] = hid_t[:, sl].reshape(IT, P, BL).transpose(1, 0, 2)
        enc_pack = np.zeros((HIN, TCOLS), dtype=F8)
        mneg = np.full(TCOLS, -1e30, dtype=np.float32)
        for b in range(BL):
            ix = idx_all[c * BL + b]
            enc_pack[:, b * CB:b * CB + len(ix)] = enc_t[c * BL + b][:, ix]
            mneg[b * CB:b * CB + len(ix)] = 0.0
        in_maps.append({
            "enc": np.ascontiguousarray(enc_pack.reshape(IT, P, TCOLS)),
            "hiddent": hid_pack,
            "maskneg": mneg,
            "w1e": w1e_pack,
            "w1h": w1h_pack,
            "b1": b1,
            "w2": w2_pack,
        })

    res = run_bass_kernel_spmd(nc, in_maps, core_ids=list(range(N_CORES)))
    LAST_RESULT = res
    out = np.zeros((B, S), dtype=np.float32)
    for c in range(N_CORES):
        packed = res.results[c]["out"]
        for b in range(BL):
            gb = c * BL + b
            ix = idx_all[gb]
            out[gb, ix] = packed[b * CB:b * CB + len(ix)]
    return np.ascontiguousarray(out[:, None, :])
